# revision 1
# baseline (speedup 1.0000x reference)
"""KNN graph kernel for Trainium2 (8 NeuronCores, Bass/Tile).

Problem: per-batch 32-NN of 16384 queries against 16384 refs (B=4 batches,
both sorted by batch id). Output matches jax reference:
  e_ref  [M*32] int32  - nearest ref indices, ascending distance per query
  e_query[M*32] int32  - repeat(arange(M), 32)
  mask   [M*32] bool   - (q_z - r_z) >= -1e-5 per edge

Strategy: queries are row-sharded across 8 cores in blocks of 128, grouped by
batch so each block only scans its own batch's refs (a padded window of
W=4264 columns instead of all 16384).  On each core, the tensor engine
computes scores s = 2*q.r - |r|^2 (= -d2 + |q|^2, order-equivalent to -d2)
for a [128 x 328] strip in PSUM; the scalar engine copies it to SBUF, the
vector engine extracts the top-8 of each 328-wide chunk (max8 + max_index),
and a second on-chip stage merges the 13x8 candidates into a per-query
top-32 (values + positions).  The host maps window-local winners back to
global ref indices and exactly recomputes the rare rows where a chunk's 8
extracted values could conceal a 9th top-32 member, or where adjacent winner
values (including the host-derived 33rd-best candidate) are too close to
trust fp ordering.  Cost-model timeline: ~228 us per core, DVE-bound
(stage-1 scans), with PE/ACT/DMA fully overlapped.
"""

import numpy as np

K = 32
P = 128          # queries per block (SBUF partitions)
CHUNK = 328      # columns per matmul strip == per max8 chunk
NCHUNK = 13
W = CHUNK * NCHUNK   # 4480-wide ref window per batch
NCAND = NCHUNK * 8   # 112 stage-1 candidates per query
NWIN = 32            # stage-2 winners extracted on-device (32; 33rd from c_val on host)
N_CORES = 8
NBLK = 17            # query blocks per core (8*17*128 = 17408 >= 16384 + pad)
SENT = -1.0e9        # sentinel score for padded window columns
STAGE1_FROM_SBUF = True  # stage-1 max8 reads ACT-copied SBUF strip vs PSUM
TAU_CHUNK = 0.05     # suspect margin for chunk-conceals-9th test
TAU_TIE = 0.02       # suspect margin for adjacent-winner near-ties

_CACHE = {}


def _np_exact_rows(q_rows_bxyz, ref_bxyz):
    """Reference-exact (f32) top-K ref indices for the given query rows."""
    rb, rx = ref_bxyz[:, 0], ref_bxyz[:, 1:4]
    qb, qx = q_rows_bxyz[:, 0], q_rows_bxyz[:, 1:4]
    d2 = (np.sum(qx * qx, axis=1)[:, None]
          + np.sum(rx * rx, axis=1)[None, :]
          - np.float32(2.0) * (qx @ rx.T)).astype(np.float32)
    d2[qb[:, None] != rb[None, :]] = np.inf
    return np.argsort(d2, axis=1, kind="stable")[:, :K].astype(np.int32)


def _np_fallback(ref_bxyz, query_bxyz):
    M = query_bxyz.shape[0]
    e_ref = np.empty((M, K), np.int32)
    step = 2048
    for s in range(0, M, step):
        e_ref[s:s + step] = _np_exact_rows(query_bxyz[s:s + step], ref_bxyz)
    return e_ref.reshape(-1)


def _build_program():
    import concourse.mybir as mybir
    import concourse.tile as tile
    from concourse import bacc

    nc = bacc.Bacc("TRN2", target_bir_lowering=False, debug=False, num_devices=1)
    f32, u32 = mybir.dt.float32, mybir.dt.uint32

    qT = nc.dram_tensor("qT", [NBLK, 4, P], f32, kind="ExternalInput").ap()
    rslab = nc.dram_tensor("rslab", [NBLK, 4, W], f32, kind="ExternalInput").ap()
    c_val = nc.dram_tensor("c_val", [NBLK * P, NCAND], f32, kind="ExternalOutput").ap()
    c_idx = nc.dram_tensor("c_idx", [NBLK * P, NCAND], u32, kind="ExternalOutput").ap()
    w_val = nc.dram_tensor("w_val", [NBLK * P, NWIN], f32, kind="ExternalOutput").ap()
    w_pos = nc.dram_tensor("w_pos", [NBLK * P, K], u32, kind="ExternalOutput").ap()

    with tile.TileContext(nc) as tc:
        with tc.tile_pool(name="qp", bufs=3) as qpool, \
             tc.tile_pool(name="rp", bufs=3) as rpool, \
             tc.tile_pool(name="cp", bufs=4) as cpool, \
             tc.tile_pool(name="wp", bufs=3) as wpool, \
             tc.tile_pool(name="ps", bufs=6, space="PSUM") as ppool:
            for blk in range(NBLK):
                qt = qpool.tile([4, P], f32)
                nc.sync.dma_start(out=qt[:], in_=qT[blk])
                rs = rpool.tile([4, W], f32)
                nc.sync.dma_start(out=rs[:], in_=rslab[blk])

                cv = cpool.tile([P, NCAND], f32, tag="cv")
                ci = cpool.tile([P, NCAND], u32, tag="ci")
                for s in range(NCHUNK):
                    ps = ppool.tile([P, CHUNK], f32)
                    nc.tensor.matmul(ps[:], qt[:], rs[:, s * CHUNK:(s + 1) * CHUNK],
                                     start=True, stop=True)
                    if STAGE1_FROM_SBUF:
                        sb = cpool.tile([P, CHUNK], f32, tag="sb")
                        nc.scalar.copy(sb[:], ps[:])
                        src = sb
                    else:
                        src = ps
                    nc.vector.max(out=cv[:, 8 * s:8 * s + 8], in_=src[:])
                    nc.vector.max_index(out=ci[:, 8 * s:8 * s + 8],
                                        in_max=cv[:, 8 * s:8 * s + 8], in_values=src[:])
                nc.sync.dma_start(out=c_val[blk * P:(blk + 1) * P], in_=cv[:])
                nc.sync.dma_start(out=c_idx[blk * P:(blk + 1) * P], in_=ci[:])

                # stage 2: 5 rounds of top-8 over the candidates.  Round 1
                # reads cv and match_replace writes a fresh cw (no pre-copy);
                # the last round extracts values only (positions of winners
                # 33..40 are never used -- only w_val[:, 32] for gap checks).
                cw = cpool.tile([P, NCAND], f32, tag="cw")
                wv = wpool.tile([P, NWIN], f32, tag="wv")
                wpi = wpool.tile([P, NWIN], u32, tag="wpi")
                nrounds = NWIN // 8
                for r in range(nrounds):
                    src2 = cv if r == 0 else cw
                    nc.vector.max(out=wv[:, 8 * r:8 * r + 8], in_=src2[:])
                    nc.vector.max_index(out=wpi[:, 8 * r:8 * r + 8],
                                        in_max=wv[:, 8 * r:8 * r + 8], in_values=src2[:])
                    if r < nrounds - 1:
                        nc.vector.match_replace(out=cw[:], in_to_replace=wv[:, 8 * r:8 * r + 8],
                                                in_values=src2[:], imm_value=-3.0e38)
                nc.sync.dma_start(out=w_val[blk * P:(blk + 1) * P], in_=wv[:])
                nc.sync.dma_start(out=w_pos[blk * P:(blk + 1) * P], in_=wpi[:, :K])
    nc.compile()
    return nc


def kernel(ref_bxyz: np.ndarray, query_bxyz: np.ndarray):
    ref_bxyz = np.ascontiguousarray(ref_bxyz, dtype=np.float32)
    query_bxyz = np.ascontiguousarray(query_bxyz, dtype=np.float32)
    M = query_bxyz.shape[0]
    N = ref_bxyz.shape[0]
    e_query = np.repeat(np.arange(M, dtype=np.int32), K)

    rb, qb = ref_bxyz[:, 0], query_bxyz[:, 0]
    bids = np.unique(np.concatenate([rb, qb]))
    ok = (M == 16384 and N == 16384 and len(bids) <= 8
          and np.all(np.diff(rb) >= 0) and np.all(np.diff(qb) >= 0)
          and np.all(bids == np.round(bids)))
    if ok:
        r_starts = np.searchsorted(rb, bids, side="left")
        r_ends = np.searchsorted(rb, bids, side="right")
        q_starts = np.searchsorted(qb, bids, side="left")
        q_ends = np.searchsorted(qb, bids, side="right")
        sizes_ok = all(32 <= (re - rs) <= W for rs, re in zip(r_starts, r_ends))
        nblocks = sum((qe - qs + P - 1) // P for qs, qe in zip(q_starts, q_ends) if qe > qs)
        ok = sizes_ok and nblocks <= NBLK * N_CORES
    if not ok:
        e_ref = _np_fallback(ref_bxyz, query_bxyz)
        direction = query_bxyz[e_query, 3] - ref_bxyz[e_ref, 3]
        return e_ref, e_query, (direction >= np.float32(-1e-5))

    # ---- host prep: per-batch ref slabs + per-block transposed queries ----
    nb = len(bids)
    slabs = np.empty((nb, 4, W), np.float32)
    slabs[:, :3, :] = 0.0
    slabs[:, 3, :] = -SENT  # sq_r sentinel => score = -sq_r = SENT
    for i, (rs_, re_) in enumerate(zip(r_starts, r_ends)):
        n = re_ - rs_
        rx = ref_bxyz[rs_:re_, 1:4]
        slabs[i, :3, :n] = rx.T
        slabs[i, 3, :n] = np.sum(rx * rx, axis=1)

    blocks = []  # (batch_i, q_start, nvalid)
    for i, (qs_, qe_) in enumerate(zip(q_starts, q_ends)):
        for s in range(qs_, qe_, P):
            blocks.append((i, s, min(P, qe_ - s)))
    total = NBLK * N_CORES
    blocks += [(0, 0, 0)] * (total - len(blocks))

    qT_in = np.zeros((N_CORES, NBLK, 4, P), np.float32)
    rs_in = np.empty((N_CORES, NBLK, 4, W), np.float32)
    qT_in[:, :, 3, :] = -1.0
    for k, (bi, qs_, nv) in enumerate(blocks):
        c, j = divmod(k, NBLK)
        if nv:
            qT_in[c, j, :3, :nv] = 2.0 * query_bxyz[qs_:qs_ + nv, 1:4].T
        rs_in[c, j] = slabs[bi]

    if "nc" not in _CACHE:
        _CACHE["nc"] = _build_program()
    nc = _CACHE["nc"]

    from concourse.bass_utils import run_bass_kernel_spmd
    in_maps = [{"qT": qT_in[c], "rslab": rs_in[c]} for c in range(N_CORES)]
    _CACHE["last_in_maps"] = in_maps
    res = run_bass_kernel_spmd(nc, in_maps, list(range(N_CORES)))
    _CACHE["last_results"] = res

    # ---- host post: map winners to global indices, repair suspect rows ----
    e_ref = np.empty((M, K), np.int32)
    suspect_q = []
    suspect_b = []
    for k, (bi, qs_, nv) in enumerate(blocks):
        if nv == 0:
            continue
        c, j = divmod(k, NBLK)
        r = res.results[c]
        sl = slice(j * P, j * P + nv)
        wv = r["w_val"][sl]
        wp = r["w_pos"][sl].astype(np.int64)
        cidx = r["c_idx"][sl].astype(np.int64)
        cval = r["c_val"][sl]
        top = wp[:, :K]
        local = (top >> 3) * CHUNK + np.take_along_axis(cidx, top, axis=1)
        e_ref[qs_:qs_ + nv] = (r_starts[bi] + local).astype(np.int32)
        v32 = wv[:, K - 1]
        chunk8 = cval[:, 7::8]
        bad = (chunk8 >= (v32[:, None] - TAU_CHUNK)).any(axis=1)
        gaps = wv[:, :K][:, :-1] - wv[:, :K][:, 1:]
        bad |= (gaps < TAU_TIE).any(axis=1)
        v33 = -np.partition(-cval, K, axis=1)[:, K]  # 33rd-best candidate
        bad |= (v32 - v33) < TAU_TIE
        bad |= v32 <= SENT / 2
        if bad.any():
            idx = np.nonzero(bad)[0]
            suspect_q.append(qs_ + idx)
            suspect_b.append(np.full(len(idx), bi))
    if suspect_q:
        sq = np.concatenate(suspect_q)
        sb_ = np.concatenate(suspect_b)
        for bi in np.unique(sb_):
            qsel = sq[sb_ == bi]
            # same-batch slice only: cross-batch refs are +inf in the
            # reference and each batch has >= K refs, so restricting the
            # argsort to the batch's contiguous ref range is exact.
            refs = ref_bxyz[r_starts[bi]:r_ends[bi]]
            for s in range(0, len(qsel), 4096):
                part = qsel[s:s + 4096]
                e_ref[part] = r_starts[bi] + _np_exact_rows(query_bxyz[part], refs)
    _CACHE["n_suspect"] = sum(len(s) for s in suspect_q)

    e_ref = e_ref.reshape(-1)
    direction = query_bxyz[e_query, 3] - ref_bxyz[e_ref, 3]
    return e_ref, e_query, (direction >= np.float32(-1e-5))



# revision 4
# speedup vs baseline: 3.9784x; 3.9784x over previous
"""KNN graph kernel for Trainium2 (8 NeuronCores, Bass/Tile).

Problem: per-batch 32-NN of 16384 queries against 16384 refs (B~4 batches,
both sorted by batch id).  Output matches the jax reference:
  e_ref  [M*32] int32  - nearest ref indices, ascending distance per query
  e_query[M*32] int32  - repeat(arange(M), 32)
  mask   [M*32] bool   - (q_z - r_z) >= -1e-5 per edge

Strategy (spatial windowing + quantized score/index packing):
  * Queries are grouped into 136 spatial cells of <=128 (per-batch y/z
    quantile split).  Each 128-query block only scans refs inside its
    cell's (y,z) bounding box expanded by RM=15 (<=1020 refs instead of
    the whole ~4096-ref batch).  A query's true 32NN ball fits in the
    window unless d_32 > RM; the host detects that exactly and repairs.
  * The tensor engine computes PSUM = T - c*d2 with T = 1.5*2^23: every
    PSUM value lands in [2^23, 2^24) where fp32 forces integer rounding,
    i.e. the score is quantized to 1/c for free.  ACT copies PSUM to SBUF
    subtracting T (exact); Pool (+ DVE for one chunk) adds idx/128
    (column-in-chunk index; exact while |m| <= 2^17 keeps the 24-bit
    total representable).  Packed values carry the quantized score AND
    the column index, so stage 1 is a SINGLE DVE max8 per 85-col chunk -
    no max_index pass, no on-device stage 2.
  * Host: decode candidates, rescore them exactly in f32 (same formula
    family as the reference), merge to top-32, and exactly recompute rows
    flagged for near-ties / chunk concealment / window-radius violations.
"""

import numpy as np

K = 32
P = 128            # queries per block (SBUF partitions)
W = 1024           # ref window cols (2 PSUM banks); last 4 always pad
CHUNK = 85
NCHUNK = 12        # 12*85 = 1020 data cols
NCAND = NCHUNK * 8
N_CORES = 8
NBLK = 17          # blocks per core -> 136 cells total
RM = 15.0          # window margin (covering-radius guarantee)
CSC = 256.0        # score scale: quantum = 1/CSC in d2 units
TBIG = 1.5 * 2**23  # 12582912; PSUM offset forcing integer quantization
MVALID = -131072.0  # candidates with m < MVALID (d2 > 512) are discarded
TAU_ORDER = 0.012  # adjacent-gap margin for exact-order trust (d2 units)
PAD_RR = 1.0e6     # |r|^2 for pad columns -> huge negative score

_CACHE = {}


def _np_exact_rows(q_rows_bxyz, ref_bxyz):
    """Reference-exact (f32) top-K ref indices for the given query rows."""
    rb, rx = ref_bxyz[:, 0], ref_bxyz[:, 1:4]
    qb, qx = q_rows_bxyz[:, 0], q_rows_bxyz[:, 1:4]
    d2 = (np.sum(qx * qx, axis=1)[:, None]
          + np.sum(rx * rx, axis=1)[None, :]
          - np.float32(2.0) * (qx @ rx.T)).astype(np.float32)
    d2[qb[:, None] != rb[None, :]] = np.inf
    return np.argsort(d2, axis=1, kind="stable")[:, :K].astype(np.int32)


def _np_fallback(ref_bxyz, query_bxyz):
    M = query_bxyz.shape[0]
    e_ref = np.empty((M, K), np.int32)
    step = 2048
    for s in range(0, M, step):
        e_ref[s:s + step] = _np_exact_rows(query_bxyz[s:s + step], ref_bxyz)
    return e_ref.reshape(-1)


def _build_program():
    import concourse.mybir as mybir
    import concourse.tile as tile
    from concourse import bacc

    nc = bacc.Bacc("TRN2", target_bir_lowering=False, debug=False, num_devices=1)
    f32, u32 = mybir.dt.float32, mybir.dt.uint32
    Copy = mybir.ActivationFunctionType.Copy

    qT = nc.dram_tensor("qT", [NBLK, 5, P], f32, kind="ExternalInput").ap()
    rslab = nc.dram_tensor("rslab", [NBLK, 5, W], f32, kind="ExternalInput").ap()
    c_val = nc.dram_tensor("c_val", [NBLK * P, NCAND], f32, kind="ExternalOutput").ap()

    POOL_SPLIT = CHUNK * 11  # Pool packs cols [0:935), DVE the last chunk

    with tile.TileContext(nc) as tc:
        with tc.tile_pool(name="const", bufs=1) as cpool, \
             tc.tile_pool(name="qp", bufs=3) as qpool, \
             tc.tile_pool(name="rp", bufs=3) as rpool, \
             tc.tile_pool(name="tp", bufs=3) as tpool, \
             tc.tile_pool(name="kp", bufs=3) as kpool, \
             tc.tile_pool(name="cvp", bufs=3) as cvpool, \
             tc.tile_pool(name="ps", bufs=4, space="PSUM") as ppool:
            NREP = W // CHUNK + 1
            idxu = cpool.tile([P, NREP * CHUNK], u32)
            nc.gpsimd.iota(idxu[:], pattern=[[0, NREP], [1, CHUNK]],
                           base=0, channel_multiplier=0)
            idxrow = cpool.tile([P, W], f32)
            nc.vector.tensor_scalar_mul(idxrow[:], idxu[:, :W], 1.0 / 128.0)

            for blk in range(NBLK):
                qt = qpool.tile([5, P], f32)
                nc.sync.dma_start(out=qt[:], in_=qT[blk])
                rs = rpool.tile([5, W], f32)
                nc.sync.dma_start(out=rs[:], in_=rslab[blk])

                t = tpool.tile([P, W], f32)
                pk = kpool.tile([P, W], f32)
                for h in range(2):
                    ps = ppool.tile([P, 512], f32)
                    nc.tensor.matmul(ps[:], qt[:], rs[:, 512 * h:512 * (h + 1)],
                                     start=True, stop=True)
                    nc.scalar.activation(t[:, 512 * h:512 * (h + 1)], ps[:],
                                         Copy, bias=-float(TBIG), scale=1.0)
                nc.gpsimd.tensor_add(pk[:, :POOL_SPLIT], t[:, :POOL_SPLIT],
                                     idxrow[:, :POOL_SPLIT])
                nc.vector.tensor_add(pk[:, POOL_SPLIT:NCHUNK * CHUNK],
                                     t[:, POOL_SPLIT:NCHUNK * CHUNK],
                                     idxrow[:, POOL_SPLIT:NCHUNK * CHUNK])
                cv = cvpool.tile([P, NCAND], f32)
                for s in range(NCHUNK):
                    nc.vector.max(out=cv[:, 8 * s:8 * s + 8],
                                  in_=pk[:, CHUNK * s:CHUNK * (s + 1)])
                nc.sync.dma_start(out=c_val[blk * P:(blk + 1) * P], in_=cv[:])
    nc.compile()
    return nc


def _apportion(total, weights):
    """Split integer `total` proportionally to weights (largest remainder)."""
    w = np.asarray(weights, dtype=np.float64)
    if w.sum() <= 0:
        out = np.zeros(len(w), np.int64)
        if len(w):
            out[0] = total
        return out
    want = w / w.sum() * total
    out = np.floor(want).astype(np.int64)
    rem = int(total - out.sum())
    order = np.argsort(-(want - out))
    out[order[:rem]] += 1
    return out


def _plan_cells(nq_per_batch):
    """Apportion NBLK*N_CORES cells across batches, >= ceil(nq/P) each."""
    total = NBLK * N_CORES
    nq = np.asarray(nq_per_batch, dtype=np.int64)
    mins = -(-nq // P)
    if mins.sum() > total:
        return None
    extra = total - int(mins.sum())
    add = _apportion(extra, nq) if extra > 0 else np.zeros(len(nq), np.int64)
    ncells = mins + add
    ncells[nq == 0] = 0
    return ncells


def kernel(ref_bxyz: np.ndarray, query_bxyz: np.ndarray):
    ref_bxyz = np.ascontiguousarray(ref_bxyz, dtype=np.float32)
    query_bxyz = np.ascontiguousarray(query_bxyz, dtype=np.float32)
    M = query_bxyz.shape[0]
    e_query = np.repeat(np.arange(M, dtype=np.int32), K)

    def finish(e_ref_flat):
        direction = query_bxyz[e_query, 3] - ref_bxyz[e_ref_flat, 3]
        return e_ref_flat, e_query, (direction >= np.float32(-1e-5))

    rb, qb = ref_bxyz[:, 0], query_bxyz[:, 0]
    bids = np.unique(np.concatenate([rb, qb]))
    ok = (len(bids) <= NBLK * N_CORES
          and np.all(np.diff(rb) >= 0) and np.all(np.diff(qb) >= 0))
    if ok:
        r_starts = np.searchsorted(rb, bids, side="left")
        r_ends = np.searchsorted(rb, bids, side="right")
        q_starts = np.searchsorted(qb, bids, side="left")
        q_ends = np.searchsorted(qb, bids, side="right")
        nq_b = q_ends - q_starts
        nr_b = r_ends - r_starts
        ok = bool(np.all((nr_b >= K) | (nq_b == 0)))
        ncells = _plan_cells(nq_b) if ok else None
        ok = ok and ncells is not None
    if not ok:
        return finish(_np_fallback(ref_bxyz, query_bxyz))

    # ---- host prep: spatial cells, windows, slabs ----
    nb = len(bids)
    total_blocks = NBLK * N_CORES
    WDAT = NCHUNK * CHUNK
    qT_in = np.zeros((total_blocks, 5, P), np.float32)
    rs_in = np.empty((total_blocks, 5, W), np.float32)
    rs_in[:, 0:3, :] = 0.0
    rs_in[:, 3, :] = PAD_RR
    rs_in[:, 4, :] = 1.0
    qT_in[:, 4, :] = np.float32(TBIG)  # pad queries: q~ = 0

    win_idx = np.full((total_blocks, W), -1, np.int64)
    rm_blk = np.full(total_blocks, np.float64(RM))
    blk_q = [None] * total_blocks  # global query indices per block

    blk = 0
    cell_overflow = False
    for bi in range(nb):
        n_b = int(ncells[bi])
        if n_b == 0:
            continue
        qs_, qe_ = int(q_starts[bi]), int(q_ends[bi])
        rs_, re_ = int(r_starts[bi]), int(r_ends[bi])
        q_idx = np.arange(qs_, qe_)
        qy, qz = query_bxyz[qs_:qe_, 2], query_bxyz[qs_:qe_, 3]
        r_xyz = ref_bxyz[rs_:re_, 1:4]
        ry, rz = r_xyz[:, 1], r_xyz[:, 2]
        cx0 = float(r_xyz[:, 0].min() + r_xyz[:, 0].max()) / 2

        gy = 4 if n_b >= 8 else 1
        gz_per = _apportion(n_b, np.ones(gy))
        mq_per = _apportion(len(q_idx), gz_per)
        y_order = np.argsort(qy, kind="stable")
        gstart = 0
        for g in range(gy):
            gsel = y_order[gstart:gstart + int(mq_per[g])]
            gstart += int(mq_per[g])
            if len(gsel) == 0:
                continue
            z_order = gsel[np.argsort(qz[gsel], kind="stable")]
            for part in np.array_split(z_order, int(gz_per[g])):
                if len(part) == 0:
                    continue
                if len(part) > P:
                    cell_overflow = True
                    break
                cq = q_idx[part]  # global query ids, cell order
                cy, cz = qy[part], qz[part]
                ymin, ymax = float(cy.min()), float(cy.max())
                zmin, zmax = float(cz.min()), float(cz.max())
                need_y = np.maximum(ymin - ry, ry - ymax)
                need_z = np.maximum(zmin - rz, rz - zmax)
                need = np.maximum(np.maximum(need_y, need_z), 0.0)
                sel = np.nonzero(need <= RM)[0]
                rmb = RM
                if len(sel) > WDAT:
                    kept = np.argpartition(need, WDAT - 1)[:WDAT]
                    sel = np.sort(kept)
                    rmb = float(need[sel].max()) - 1e-4
                nw = len(sel)
                win_idx[blk, :nw] = rs_ + sel
                rm_blk[blk] = rmb
                blk_q[blk] = cq
                c0 = np.array([cx0, (ymin + ymax) / 2, (zmin + zmax) / 2],
                              np.float64)
                rt = (r_xyz[sel].astype(np.float64) - c0).astype(np.float32)
                rs_in[blk, 0:3, :nw] = rt.T
                rs_in[blk, 3, :nw] = np.sum(
                    rt.astype(np.float64) ** 2, axis=1).astype(np.float32)
                qt_ = (query_bxyz[cq, 1:4].astype(np.float64) - c0).astype(
                    np.float32)
                nv = len(cq)
                qT_in[blk, 0:3, :nv] = (2.0 * CSC) * qt_.T
                qT_in[blk, 3, :] = np.float32(-CSC)
                qT_in[blk, 4, :nv] = (
                    TBIG - CSC * np.sum(qt_.astype(np.float64) ** 2, axis=1)
                ).astype(np.float32)
                blk += 1
            if cell_overflow:
                break
        if cell_overflow:
            break
    if cell_overflow:
        return finish(_np_fallback(ref_bxyz, query_bxyz))

    # ---- device ----
    if "nc" not in _CACHE:
        _CACHE["nc"] = _build_program()
    nc = _CACHE["nc"]
    from concourse.bass_utils import run_bass_kernel_spmd
    qT_c = qT_in.reshape(N_CORES, NBLK, 5, P)
    rs_c = rs_in.reshape(N_CORES, NBLK, 5, W)
    in_maps = [{"qT": qT_c[c], "rslab": rs_c[c]} for c in range(N_CORES)]
    _CACHE["last_in_maps"] = in_maps
    res = run_bass_kernel_spmd(nc, in_maps, list(range(N_CORES)))
    _CACHE["last_results"] = res

    cv = np.concatenate([res.results[c]["c_val"] for c in range(N_CORES)],
                        axis=0).reshape(total_blocks, P, NCAND)

    # ---- host post: decode, rescore, merge, repair ----
    blocks_list = [i for i in range(total_blocks) if blk_q[i] is not None]
    bsel = np.concatenate([np.full(len(blk_q[i]), i, np.int64)
                           for i in blocks_list])
    rsel = np.concatenate([np.arange(len(blk_q[i]), dtype=np.int64)
                           for i in blocks_list])
    q_flat = np.concatenate([blk_q[i] for i in blocks_list])  # [M] global qids

    p = cv[bsel, rsel].astype(np.float64)              # [M, NCAND]
    m = np.floor(p)
    idxl = np.rint((p - m) * 128.0).astype(np.int64)
    pos = (np.arange(NCAND) // 8)[None, :] * CHUNK + np.clip(idxl, 0, CHUNK - 1)
    valid = (m >= MVALID) & (idxl < CHUNK)
    d2q = -m / CSC                                     # device-quantized d2
    gidx = win_idx[bsel[:, None], pos]
    valid &= gidx >= 0
    gidx_c = np.where(valid, gidx, 0)

    qx_all = query_bxyz[:, 1:4]
    rx_all = ref_bxyz[:, 1:4]
    qq_all = np.sum(qx_all * qx_all, axis=1)           # f32, reference formula
    rr_all = np.sum(rx_all * rx_all, axis=1)

    dot = np.einsum("qd,qkd->qk", qx_all[q_flat], rx_all[gidx_c],
                    dtype=np.float32, casting="same_kind")
    d2x = (qq_all[q_flat][:, None] + rr_all[gidx_c]
           - np.float32(2.0) * dot).astype(np.float64)
    d2x[~valid] = np.inf

    near = valid & (d2q < 500.0)
    e_obs = float(np.abs(np.where(near, d2q - d2x, 0.0)).max())
    if e_obs > 1.0:
        return finish(_np_fallback(ref_bxyz, query_bxyz))
    kappa = 2.0 * e_obs + 2.0 / CSC + 1e-3

    order = np.lexsort((gidx_c, d2x), axis=-1)
    top = order[:, :K + 1]
    d2_sorted = np.take_along_axis(d2x, top, axis=1)
    ref_sorted = np.take_along_axis(gidx_c, top, axis=1)

    e_ref = np.empty((M, K), np.int32)
    e_ref[q_flat] = ref_sorted[:, :K].astype(np.int32)

    nvalid = valid.sum(axis=1)
    d2_32 = d2_sorted[:, K - 1]
    bad = nvalid < K + 1
    bad |= d2_32 > (rm_blk[bsel] - 1e-3) ** 2
    bad |= (np.diff(d2_sorted, axis=1) < TAU_ORDER).any(axis=1)
    ch8 = d2q[:, 7::8]
    bad |= (valid[:, 7::8] & (ch8 <= d2_32[:, None] + kappa)).any(axis=1)

    if bad.any():
        sq = q_flat[bad]
        sb_ = np.searchsorted(q_starts, sq, side="right") - 1
        for bi in np.unique(sb_):
            qsel = sq[sb_ == bi]
            refs = ref_bxyz[r_starts[bi]:r_ends[bi]]
            for s in range(0, len(qsel), 4096):
                part = qsel[s:s + 4096]
                e_ref[part] = r_starts[bi] + _np_exact_rows(
                    query_bxyz[part], refs)
    _CACHE["n_suspect"] = int(bad.sum())
    _CACHE["e_obs"] = e_obs

    return finish(e_ref.reshape(-1))


# revision 8
# speedup vs baseline: 4.3377x; 1.0903x over previous
"""KNN graph kernel for Trainium2 (8 NeuronCores, Bass/Tile).

Problem: per-batch 32-NN of 16384 queries against 16384 refs (B~4 batches,
both sorted by batch id).  Output matches the jax reference:
  e_ref  [M*32] int32  - nearest ref indices, ascending distance per query
  e_query[M*32] int32  - repeat(arange(M), 32)
  mask   [M*32] bool   - (q_z - r_z) >= -1e-5 per edge

Strategy (spatial windowing + quantized score/index packing):
  * Queries are grouped into 136 spatial cells of <=128 (per-batch y/z
    quantile split).  Each 128-query block only scans refs inside its
    cell's (y,z) bounding box expanded by RM=15 (<=1020 refs instead of
    the whole ~4096-ref batch).  A query's true 32NN ball fits in the
    window unless d_32 > RM; the host detects that exactly and repairs.
  * The tensor engine computes PSUM = T - c*d2 with T = 1.5*2^23: every
    PSUM value lands in [2^23, 2^24) where fp32 forces integer rounding,
    i.e. the score is quantized to 1/c for free.  ACT copies PSUM to SBUF
    subtracting T (exact); Pool (+ DVE for one chunk) adds idx/128
    (column-in-chunk index; exact while |m| <= 2^17 keeps the 24-bit
    total representable).  Packed values carry the quantized score AND
    the column index, so stage 1 is a SINGLE DVE max8 per 85-col chunk -
    no max_index pass, no on-device stage 2.
  * Host: decode candidates, rescore them exactly in f32 (same formula
    family as the reference), merge to top-32, and exactly recompute rows
    flagged for near-ties / chunk concealment / window-radius violations.
"""

import numpy as np

K = 32
P = 128            # queries per block (SBUF partitions)
W = 1024           # ref window cols (2 PSUM banks); last 4 always pad
CHUNK = 85
NCHUNK = 12        # 12*85 = 1020 data cols
NCAND = NCHUNK * 8
N_CORES = 8
NBLK = 17          # blocks per core -> 136 cells total
RM = 15.0          # window margin (covering-radius guarantee)
CSC = 256.0        # score scale: quantum = 1/CSC in d2 units
TBIG = 1.5 * 2**23  # 12582912; PSUM offset forcing integer quantization
MVALID = -131072.0  # candidates with m < MVALID (d2 > 512) are discarded
TAU_ORDER = 0.012  # adjacent-gap margin for exact-order trust (d2 units)
PAD_RR = 1.0e6     # |r|^2 for pad columns -> huge negative score

_CACHE = {}


def _np_exact_rows(q_rows_bxyz, ref_bxyz):
    """Reference-exact (f32) top-K ref indices for the given query rows."""
    rb, rx = ref_bxyz[:, 0], ref_bxyz[:, 1:4]
    qb, qx = q_rows_bxyz[:, 0], q_rows_bxyz[:, 1:4]
    d2 = (np.sum(qx * qx, axis=1)[:, None]
          + np.sum(rx * rx, axis=1)[None, :]
          - np.float32(2.0) * (qx @ rx.T)).astype(np.float32)
    d2[qb[:, None] != rb[None, :]] = np.inf
    return np.argsort(d2, axis=1, kind="stable")[:, :K].astype(np.int32)


def _np_fallback(ref_bxyz, query_bxyz):
    M = query_bxyz.shape[0]
    e_ref = np.empty((M, K), np.int32)
    step = 2048
    for s in range(0, M, step):
        e_ref[s:s + step] = _np_exact_rows(query_bxyz[s:s + step], ref_bxyz)
    return e_ref.reshape(-1)


def _build_program():
    import concourse.mybir as mybir
    import concourse.tile as tile
    from concourse import bacc

    nc = bacc.Bacc("TRN2", target_bir_lowering=False, debug=False, num_devices=1)
    f32, u32 = mybir.dt.float32, mybir.dt.uint32
    Copy = mybir.ActivationFunctionType.Copy

    qT = nc.dram_tensor("qT", [NBLK, 5, P], f32, kind="ExternalInput").ap()
    rslab = nc.dram_tensor("rslab", [NBLK, 5, W], f32, kind="ExternalInput").ap()
    c_val = nc.dram_tensor("c_val", [NBLK * P, NCAND], f32, kind="ExternalOutput").ap()

    HALF = NCHUNK * CHUNK // 2  # 510 cols per half (one PSUM bank each)
    DVE_COLS = 50               # trailing cols per half packed by DVE (balance)

    with tile.TileContext(nc) as tc:
        with tc.tile_pool(name="const", bufs=1) as cpool, \
             tc.tile_pool(name="qp", bufs=4) as qpool, \
             tc.tile_pool(name="rp", bufs=4) as rpool, \
             tc.tile_pool(name="tp", bufs=4) as tpool, \
             tc.tile_pool(name="kp", bufs=4) as kpool, \
             tc.tile_pool(name="cvp", bufs=4) as cvpool, \
             tc.tile_pool(name="ps", bufs=6, space="PSUM") as ppool:
            NREP = W // CHUNK + 1
            idxu = cpool.tile([P, NREP * CHUNK], u32)
            nc.gpsimd.iota(idxu[:], pattern=[[0, NREP], [1, CHUNK]],
                           base=0, channel_multiplier=0)
            idxrow = cpool.tile([P, W], f32)
            nc.vector.tensor_scalar_mul(idxrow[:], idxu[:, :W], 1.0 / 128.0)

            for blk in range(NBLK):
                qt = qpool.tile([5, P], f32)
                nc.sync.dma_start(out=qt[:], in_=qT[blk])
                rs = rpool.tile([5, W], f32)
                nc.sync.dma_start(out=rs[:], in_=rslab[blk])

                t = tpool.tile([P, W], f32)
                pk = kpool.tile([P, W], f32)
                cv = cvpool.tile([P, NCAND], f32)
                # two independent halves -> matmul/ACT/pack/max8 pipeline
                for h in range(2):
                    lo = HALF * h
                    ps = ppool.tile([P, HALF], f32)
                    nc.tensor.matmul(ps[:], qt[:], rs[:, lo:lo + HALF],
                                     start=True, stop=True)
                    nc.scalar.activation(t[:, lo:lo + HALF], ps[:],
                                         Copy, bias=-float(TBIG), scale=1.0)
                    csplit = lo + HALF - DVE_COLS
                    nc.gpsimd.tensor_add(pk[:, lo:csplit], t[:, lo:csplit],
                                         idxrow[:, lo:csplit])
                    nc.vector.tensor_add(pk[:, csplit:lo + HALF],
                                         t[:, csplit:lo + HALF],
                                         idxrow[:, csplit:lo + HALF])
                    for s in range(NCHUNK // 2 * h, NCHUNK // 2 * (h + 1)):
                        nc.vector.max(out=cv[:, 8 * s:8 * s + 8],
                                      in_=pk[:, CHUNK * s:CHUNK * (s + 1)])
                nc.sync.dma_start(out=c_val[blk * P:(blk + 1) * P], in_=cv[:])
    nc.compile()
    return nc


def _apportion(total, weights):
    """Split integer `total` proportionally to weights (largest remainder)."""
    w = np.asarray(weights, dtype=np.float64)
    if w.sum() <= 0:
        out = np.zeros(len(w), np.int64)
        if len(w):
            out[0] = total
        return out
    want = w / w.sum() * total
    out = np.floor(want).astype(np.int64)
    rem = int(total - out.sum())
    order = np.argsort(-(want - out))
    out[order[:rem]] += 1
    return out


def _plan_cells(nq_per_batch):
    """Apportion NBLK*N_CORES cells across batches, >= ceil(nq/P) each."""
    total = NBLK * N_CORES
    nq = np.asarray(nq_per_batch, dtype=np.int64)
    mins = -(-nq // P)
    if mins.sum() > total:
        return None
    extra = total - int(mins.sum())
    add = _apportion(extra, nq) if extra > 0 else np.zeros(len(nq), np.int64)
    ncells = mins + add
    ncells[nq == 0] = 0
    return ncells


def kernel(ref_bxyz: np.ndarray, query_bxyz: np.ndarray):
    ref_bxyz = np.ascontiguousarray(ref_bxyz, dtype=np.float32)
    query_bxyz = np.ascontiguousarray(query_bxyz, dtype=np.float32)
    M = query_bxyz.shape[0]
    e_query = np.repeat(np.arange(M, dtype=np.int32), K)

    def finish(e_ref_flat):
        direction = query_bxyz[e_query, 3] - ref_bxyz[e_ref_flat, 3]
        return e_ref_flat, e_query, (direction >= np.float32(-1e-5))

    rb, qb = ref_bxyz[:, 0], query_bxyz[:, 0]
    bids = np.unique(np.concatenate([rb, qb]))
    ok = (len(bids) <= NBLK * N_CORES
          and np.all(np.diff(rb) >= 0) and np.all(np.diff(qb) >= 0))
    if ok:
        r_starts = np.searchsorted(rb, bids, side="left")
        r_ends = np.searchsorted(rb, bids, side="right")
        q_starts = np.searchsorted(qb, bids, side="left")
        q_ends = np.searchsorted(qb, bids, side="right")
        nq_b = q_ends - q_starts
        nr_b = r_ends - r_starts
        ok = bool(np.all((nr_b >= K) | (nq_b == 0)))
        ncells = _plan_cells(nq_b) if ok else None
        ok = ok and ncells is not None
    if not ok:
        return finish(_np_fallback(ref_bxyz, query_bxyz))

    # ---- host prep: spatial cells, windows, slabs ----
    nb = len(bids)
    total_blocks = NBLK * N_CORES
    WDAT = NCHUNK * CHUNK
    qT_in = np.zeros((total_blocks, 5, P), np.float32)
    rs_in = np.empty((total_blocks, 5, W), np.float32)
    rs_in[:, 0:3, :] = 0.0
    rs_in[:, 3, :] = PAD_RR
    rs_in[:, 4, :] = 1.0
    qT_in[:, 4, :] = np.float32(TBIG)  # pad queries: q~ = 0

    win_idx = np.full((total_blocks, W), -1, np.int64)
    rm_blk = np.full(total_blocks, np.float64(RM))
    blk_q = [None] * total_blocks  # global query indices per block

    blk = 0
    cell_overflow = False
    for bi in range(nb):
        n_b = int(ncells[bi])
        if n_b == 0:
            continue
        qs_, qe_ = int(q_starts[bi]), int(q_ends[bi])
        rs_, re_ = int(r_starts[bi]), int(r_ends[bi])
        q_idx = np.arange(qs_, qe_)
        qy, qz = query_bxyz[qs_:qe_, 2], query_bxyz[qs_:qe_, 3]
        r_xyz = ref_bxyz[rs_:re_, 1:4]
        ry, rz = r_xyz[:, 1], r_xyz[:, 2]
        cx0 = float(r_xyz[:, 0].min() + r_xyz[:, 0].max()) / 2

        gy = 4 if n_b >= 8 else 1
        gz_per = _apportion(n_b, np.ones(gy))
        mq_per = _apportion(len(q_idx), gz_per)
        y_order = np.argsort(qy, kind="stable")
        gstart = 0
        for g in range(gy):
            gsel = y_order[gstart:gstart + int(mq_per[g])]
            gstart += int(mq_per[g])
            if len(gsel) == 0:
                continue
            z_order = gsel[np.argsort(qz[gsel], kind="stable")]
            for part in np.array_split(z_order, int(gz_per[g])):
                if len(part) == 0:
                    continue
                if len(part) > P:
                    cell_overflow = True
                    break
                cq = q_idx[part]  # global query ids, cell order
                cy, cz = qy[part], qz[part]
                ymin, ymax = float(cy.min()), float(cy.max())
                zmin, zmax = float(cz.min()), float(cz.max())
                need_y = np.maximum(ymin - ry, ry - ymax)
                need_z = np.maximum(zmin - rz, rz - zmax)
                need = np.maximum(np.maximum(need_y, need_z), 0.0)
                sel = np.nonzero(need <= RM)[0]
                rmb = RM
                if len(sel) > WDAT:
                    kept = np.argpartition(need, WDAT - 1)[:WDAT]
                    sel = np.sort(kept)
                    rmb = float(need[sel].max()) - 1e-4
                nw = len(sel)
                win_idx[blk, :nw] = rs_ + sel
                rm_blk[blk] = rmb
                blk_q[blk] = cq
                c0 = np.array([cx0, (ymin + ymax) / 2, (zmin + zmax) / 2],
                              np.float64)
                rt = (r_xyz[sel].astype(np.float64) - c0).astype(np.float32)
                rs_in[blk, 0:3, :nw] = rt.T
                rs_in[blk, 3, :nw] = np.sum(
                    rt.astype(np.float64) ** 2, axis=1).astype(np.float32)
                qt_ = (query_bxyz[cq, 1:4].astype(np.float64) - c0).astype(
                    np.float32)
                nv = len(cq)
                qT_in[blk, 0:3, :nv] = (2.0 * CSC) * qt_.T
                qT_in[blk, 3, :] = np.float32(-CSC)
                qT_in[blk, 4, :nv] = (
                    TBIG - CSC * np.sum(qt_.astype(np.float64) ** 2, axis=1)
                ).astype(np.float32)
                blk += 1
            if cell_overflow:
                break
        if cell_overflow:
            break
    if cell_overflow:
        return finish(_np_fallback(ref_bxyz, query_bxyz))

    # ---- device ----
    if "nc" not in _CACHE:
        _CACHE["nc"] = _build_program()
    nc = _CACHE["nc"]
    from concourse.bass_utils import run_bass_kernel_spmd
    qT_c = qT_in.reshape(N_CORES, NBLK, 5, P)
    rs_c = rs_in.reshape(N_CORES, NBLK, 5, W)
    in_maps = [{"qT": qT_c[c], "rslab": rs_c[c]} for c in range(N_CORES)]
    _CACHE["last_in_maps"] = in_maps
    res = run_bass_kernel_spmd(nc, in_maps, list(range(N_CORES)))
    _CACHE["last_results"] = res

    cv = np.concatenate([res.results[c]["c_val"] for c in range(N_CORES)],
                        axis=0).reshape(total_blocks, P, NCAND)

    # ---- host post: decode, rescore, merge, repair ----
    blocks_list = [i for i in range(total_blocks) if blk_q[i] is not None]
    bsel = np.concatenate([np.full(len(blk_q[i]), i, np.int64)
                           for i in blocks_list])
    rsel = np.concatenate([np.arange(len(blk_q[i]), dtype=np.int64)
                           for i in blocks_list])
    q_flat = np.concatenate([blk_q[i] for i in blocks_list])  # [M] global qids

    p = cv[bsel, rsel].astype(np.float64)              # [M, NCAND]
    m = np.floor(p)
    idxl = np.rint((p - m) * 128.0).astype(np.int64)
    pos = (np.arange(NCAND) // 8)[None, :] * CHUNK + np.clip(idxl, 0, CHUNK - 1)
    valid = (m >= MVALID) & (idxl < CHUNK)
    d2q = -m / CSC                                     # device-quantized d2
    gidx = win_idx[bsel[:, None], pos]
    valid &= gidx >= 0
    gidx_c = np.where(valid, gidx, 0)

    qx_all = query_bxyz[:, 1:4]
    rx_all = ref_bxyz[:, 1:4]
    qq_all = np.sum(qx_all * qx_all, axis=1)           # f32, reference formula
    rr_all = np.sum(rx_all * rx_all, axis=1)

    dot = np.einsum("qd,qkd->qk", qx_all[q_flat], rx_all[gidx_c],
                    dtype=np.float32, casting="same_kind")
    d2x = (qq_all[q_flat][:, None] + rr_all[gidx_c]
           - np.float32(2.0) * dot).astype(np.float64)
    d2x[~valid] = np.inf

    near = valid & (d2q < 500.0)
    e_obs = float(np.abs(np.where(near, d2q - d2x, 0.0)).max())
    if e_obs > 1.0:
        return finish(_np_fallback(ref_bxyz, query_bxyz))
    kappa = 2.0 * e_obs + 2.0 / CSC + 1e-3

    order = np.lexsort((gidx_c, d2x), axis=-1)
    top = order[:, :K + 1]
    d2_sorted = np.take_along_axis(d2x, top, axis=1)
    ref_sorted = np.take_along_axis(gidx_c, top, axis=1)

    e_ref = np.empty((M, K), np.int32)
    e_ref[q_flat] = ref_sorted[:, :K].astype(np.int32)

    nvalid = valid.sum(axis=1)
    d2_32 = d2_sorted[:, K - 1]
    b_nv = nvalid < K + 1
    b_vio = d2_32 > (rm_blk[bsel] - 1e-3) ** 2
    b_tie = (np.diff(d2_sorted, axis=1) < TAU_ORDER).any(axis=1)
    ch8 = d2q[:, 7::8]
    b_con = (valid[:, 7::8] & (ch8 <= d2_32[:, None] + kappa)).any(axis=1)
    bad = b_nv | b_vio | b_tie | b_con
    _CACHE["sus"] = dict(nv=int(b_nv.sum()), vio=int(b_vio.sum()),
                         tie=int(b_tie.sum()), con=int(b_con.sum()),
                         e_obs=e_obs, kappa=kappa)

    if bad.any():
        sq = q_flat[bad]
        sb_ = np.searchsorted(q_starts, sq, side="right") - 1
        for bi in np.unique(sb_):
            qsel = sq[sb_ == bi]
            refs = ref_bxyz[r_starts[bi]:r_ends[bi]]
            for s in range(0, len(qsel), 4096):
                part = qsel[s:s + 4096]
                e_ref[part] = r_starts[bi] + _np_exact_rows(
                    query_bxyz[part], refs)
    _CACHE["n_suspect"] = int(bad.sum())
    _CACHE["e_obs"] = e_obs

    return finish(e_ref.reshape(-1))
